# revision 2
# baseline (speedup 1.0000x reference)
"""Trainium2 Bass kernel for out = (x @ W) @ E.T, batch-sharded over 8 NeuronCores.

Shapes (hardcoded, full problem):
  x [4096, 2048] f32, W [2048, 300] f32, E [20000, 300] f32 -> out [4096, 20000] f32

Strategy: data-parallel over batch. Each core gets a 512-row batch shard of x
(pre-transposed on host to xT [2048, 512]), a replicated W [2048, 300], and a
replicated pre-transposed eT [300, 20000]. Host transposes mean every matmul
contracts along the SBUF partition dim with no on-chip transposes.

Per-core device kernel:
  stage 1: xwT[k, b] = sum_i W[i, k] * xT[i, b]     -> [300, 512] kept in SBUF
  stage 2: out[b, c] = sum_k xwT[k, b] * eT[k, c]   -> [512, 20000] streamed out
"""

import numpy as np

import concourse.bass as bass
import concourse.tile as tile
from concourse import bacc, mybir
from concourse.bass import ts
from concourse.bass_utils import run_bass_kernel_spmd

B, IMG, WORD, NCLS = 4096, 2048, 300, 20000
NCORES = 8
BS = B // NCORES  # 512 batch rows per core

# k (WORD) chunks on the contraction partition dim: 300 = 128 + 128 + 44
KCH = [(0, 128), (128, 128), (256, 44)]
CGRP = 2000  # classes per eT load group
CSUB = 500   # classes per matmul (N <= 512 f32 PSUM-bank limit)

# Compute dtype for matmul operands. float32 = correct baseline (4 cyc/row on PE);
# bfloat16 halves input DMA and runs PE at 1 cyc/row.
COMPUTE_DT = mybir.dt.float32


def build_nc(compute_dt=None):
    cdt = compute_dt or COMPUTE_DT
    f32 = mybir.dt.float32
    nc = bacc.Bacc(
        "TRN2",
        target_bir_lowering=False,
        debug=False,
        num_devices=NCORES,
    )
    xT = nc.declare_dram_parameter("xT", [IMG, BS], cdt, isOutput=False)
    w = nc.declare_dram_parameter("w", [IMG, WORD], cdt, isOutput=False)
    eT = nc.declare_dram_parameter("eT", [WORD, NCLS], cdt, isOutput=False)
    out = nc.declare_dram_parameter("out", [BS, NCLS], f32, isOutput=True)

    NI = IMG // 128  # 16 i-tiles

    with tile.TileContext(nc) as tc:
        with (
            tc.tile_pool(name="stage1", bufs=1) as s1_pool,
            tc.tile_pool(name="xw", bufs=1) as xw_pool,
            tc.tile_pool(name="et", bufs=2) as et_pool,
            tc.tile_pool(name="outsb", bufs=6) as out_pool,
            tc.tile_pool(name="psum", bufs=8, space="PSUM") as ps_pool,
        ):
            # ---- stage 1: xwT = W^T @ x^T^T  (contract over i=IMG)
            w_sb = s1_pool.tile([128, NI, WORD], cdt)
            x_sb = s1_pool.tile([128, NI, BS], cdt)
            nc.sync.dma_start(w_sb[:], w.rearrange("(n p) k -> p n k", p=128))
            nc.sync.dma_start(x_sb[:], xT.rearrange("(n p) b -> p n b", p=128))

            xw_sb = []
            for kc, (ko, ks) in enumerate(KCH):
                ps = ps_pool.tile([128, BS], f32, tag="ps")
                for n in range(NI):
                    nc.tensor.matmul(
                        ps[:ks, :],
                        w_sb[:, n, ko : ko + ks],
                        x_sb[:, n, :],
                        start=(n == 0),
                        stop=(n == NI - 1),
                    )
                t = xw_pool.tile([128, BS], cdt, tag=f"xw{kc}")
                nc.vector.tensor_copy(t[:ks, :], ps[:ks, :])
                xw_sb.append(t)

            # ---- stage 2: out = xwT^T @ eT  (contract over k=WORD)
            for g in range(NCLS // CGRP):
                et = et_pool.tile([128, len(KCH), CGRP], cdt, tag="et")
                for kc, (ko, ks) in enumerate(KCH):
                    nc.sync.dma_start(
                        et[:ks, kc, :], eT[ko : ko + ks, g * CGRP : (g + 1) * CGRP]
                    )
                for b in range(BS // 128):
                    ob = out_pool.tile([128, CGRP], f32, tag="ob")
                    pss = [
                        ps_pool.tile([128, CSUB], f32, tag="ps", name=f"ps2_{g}_{b}_{c}")
                        for c in range(CGRP // CSUB)
                    ]
                    # kc outer so the stationary operand is reused across c
                    for kc, (ko, ks) in enumerate(KCH):
                        for c in range(CGRP // CSUB):
                            nc.tensor.matmul(
                                pss[c][:, :],
                                xw_sb[kc][:ks, ts(b, 128)],
                                et[:ks, kc, ts(c, CSUB)],
                                start=(kc == 0),
                                stop=(kc == len(KCH) - 1),
                            )
                    for c in range(CGRP // CSUB):
                        nc.vector.tensor_copy(ob[:, ts(c, CSUB)], pss[c][:, :])
                    nc.sync.dma_start(
                        out[ts(b, 128), g * CGRP : (g + 1) * CGRP], ob[:]
                    )

    nc.compile()
    return nc


_NC_CACHE = {}


def _get_nc():
    key = str(COMPUTE_DT)
    if key not in _NC_CACHE:
        _NC_CACHE[key] = build_nc()
    return _NC_CACHE[key]


def _np_compute_dtype():
    if COMPUTE_DT == mybir.dt.bfloat16:
        import ml_dtypes

        return np.dtype(ml_dtypes.bfloat16)
    return np.dtype(np.float32)


def _prepare_in_maps(x, embedding_matrix, W):
    npdt = _np_compute_dtype()
    x = np.asarray(x, dtype=np.float32)
    E = np.asarray(embedding_matrix, dtype=np.float32)
    Wm = np.asarray(W, dtype=np.float32)
    xT = np.ascontiguousarray(x.T).astype(npdt)  # [IMG, B]
    w = np.ascontiguousarray(Wm).astype(npdt)
    eT = np.ascontiguousarray(E.T).astype(npdt)  # [WORD, NCLS]
    return [
        {
            "xT": np.ascontiguousarray(xT[:, i * BS : (i + 1) * BS]),
            "w": w,
            "eT": eT,
        }
        for i in range(NCORES)
    ]


def run(x, embedding_matrix, W, trace=False, **spmd_kwargs):
    in_maps = _prepare_in_maps(x, embedding_matrix, W)
    nc = _get_nc()
    res = run_bass_kernel_spmd(
        nc, in_maps, core_ids=list(range(NCORES)), trace=trace, **spmd_kwargs
    )
    out = np.concatenate(
        [np.asarray(res.results[i]["out"]) for i in range(NCORES)], axis=0
    )
    return out.astype(np.float32, copy=False), res


def kernel(x, embedding_matrix, W):
    out, _ = run(x, embedding_matrix, W, trace=False)
    return out


# revision 3
# speedup vs baseline: 2.5172x; 2.5172x over previous
"""Trainium2 Bass kernel for out = (x @ W) @ E.T, batch-sharded over 8 NeuronCores.

Shapes (hardcoded, full problem):
  x [4096, 2048] f32, W [2048, 300] f32, E [20000, 300] f32 -> out [4096, 20000] f32

Strategy: data-parallel over batch. Each core gets a 512-row batch shard of x
(pre-transposed on host to xT [2048, 512]), a replicated W [2048, 300], and a
replicated pre-transposed eT [300, 20000]. Host transposes mean every matmul
contracts along the SBUF partition dim with no on-chip transposes.

Per-core device kernel:
  stage 1: xwT[k, b] = sum_i W[i, k] * xT[i, b]     -> [300, 512] kept in SBUF
  stage 2: out[b, c] = sum_k xwT[k, b] * eT[k, c]   -> [512, 20000] streamed out
"""

import numpy as np

import concourse.bass as bass
import concourse.tile as tile
from concourse import bacc, mybir
from concourse.bass import ts
from concourse.bass_utils import run_bass_kernel_spmd

B, IMG, WORD, NCLS = 4096, 2048, 300, 20000
NCORES = 8
BS = B // NCORES  # 512 batch rows per core

# k (WORD) chunks on the contraction partition dim: 300 = 128 + 128 + 44
KCH = [(0, 128), (128, 128), (256, 44)]
CGRP = 2000  # classes per eT load group
CSUB = 500   # classes per matmul (N <= 512 f32 PSUM-bank limit)

# Compute dtype for matmul operands. float32 = correct baseline (4 cyc/row on PE);
# bfloat16 halves input DMA and runs PE at 1 cyc/row.
COMPUTE_DT = mybir.dt.bfloat16


def build_nc(compute_dt=None):
    cdt = compute_dt or COMPUTE_DT
    f32 = mybir.dt.float32
    nc = bacc.Bacc(
        "TRN2",
        target_bir_lowering=False,
        debug=False,
        num_devices=NCORES,
    )
    xT = nc.declare_dram_parameter("xT", [IMG, BS], cdt, isOutput=False)
    w = nc.declare_dram_parameter("w", [IMG, WORD], cdt, isOutput=False)
    eT = nc.declare_dram_parameter("eT", [WORD, NCLS], cdt, isOutput=False)
    out = nc.declare_dram_parameter("out", [BS, NCLS], f32, isOutput=True)

    NI = IMG // 128  # 16 i-tiles

    with tile.TileContext(nc) as tc:
        with (
            tc.tile_pool(name="stage1", bufs=1) as s1_pool,
            tc.tile_pool(name="xw", bufs=1) as xw_pool,
            tc.tile_pool(name="et", bufs=2) as et_pool,
            tc.tile_pool(name="outsb", bufs=6) as out_pool,
            tc.tile_pool(name="psum", bufs=8, space="PSUM") as ps_pool,
        ):
            # ---- stage 1: xwT = W^T @ x^T^T  (contract over i=IMG)
            w_sb = s1_pool.tile([128, NI, WORD], cdt)
            x_sb = s1_pool.tile([128, NI, BS], cdt)
            nc.sync.dma_start(w_sb[:], w.rearrange("(n p) k -> p n k", p=128))
            nc.sync.dma_start(x_sb[:], xT.rearrange("(n p) b -> p n b", p=128))

            xw_sb = []
            for kc, (ko, ks) in enumerate(KCH):
                ps = ps_pool.tile([128, BS], f32, tag="ps")
                for n in range(NI):
                    nc.tensor.matmul(
                        ps[:ks, :],
                        w_sb[:, n, ko : ko + ks],
                        x_sb[:, n, :],
                        start=(n == 0),
                        stop=(n == NI - 1),
                    )
                t = xw_pool.tile([128, BS], cdt, tag=f"xw{kc}")
                nc.vector.tensor_copy(t[:ks, :], ps[:ks, :])
                xw_sb.append(t)

            # ---- stage 2: out = xwT^T @ eT  (contract over k=WORD)
            for g in range(NCLS // CGRP):
                et = et_pool.tile([128, len(KCH), CGRP], cdt, tag="et")
                for kc, (ko, ks) in enumerate(KCH):
                    nc.sync.dma_start(
                        et[:ks, kc, :], eT[ko : ko + ks, g * CGRP : (g + 1) * CGRP]
                    )
                for b in range(BS // 128):
                    ob = out_pool.tile([128, CGRP], f32, tag="ob")
                    pss = [
                        ps_pool.tile([128, CSUB], f32, tag="ps", name=f"ps2_{g}_{b}_{c}")
                        for c in range(CGRP // CSUB)
                    ]
                    # kc outer so the stationary operand is reused across c
                    for kc, (ko, ks) in enumerate(KCH):
                        for c in range(CGRP // CSUB):
                            nc.tensor.matmul(
                                pss[c][:, :],
                                xw_sb[kc][:ks, ts(b, 128)],
                                et[:ks, kc, ts(c, CSUB)],
                                start=(kc == 0),
                                stop=(kc == len(KCH) - 1),
                            )
                    for c in range(CGRP // CSUB):
                        nc.vector.tensor_copy(ob[:, ts(c, CSUB)], pss[c][:, :])
                    nc.sync.dma_start(
                        out[ts(b, 128), g * CGRP : (g + 1) * CGRP], ob[:]
                    )

    nc.compile()
    return nc


_NC_CACHE = {}


def _get_nc():
    key = str(COMPUTE_DT)
    if key not in _NC_CACHE:
        _NC_CACHE[key] = build_nc()
    return _NC_CACHE[key]


def _np_compute_dtype():
    if COMPUTE_DT == mybir.dt.bfloat16:
        import ml_dtypes

        return np.dtype(ml_dtypes.bfloat16)
    return np.dtype(np.float32)


def _prepare_in_maps(x, embedding_matrix, W):
    npdt = _np_compute_dtype()
    x = np.asarray(x, dtype=np.float32)
    E = np.asarray(embedding_matrix, dtype=np.float32)
    Wm = np.asarray(W, dtype=np.float32)
    xT = np.ascontiguousarray(x.T).astype(npdt)  # [IMG, B]
    w = np.ascontiguousarray(Wm).astype(npdt)
    eT = np.ascontiguousarray(E.T).astype(npdt)  # [WORD, NCLS]
    return [
        {
            "xT": np.ascontiguousarray(xT[:, i * BS : (i + 1) * BS]),
            "w": w,
            "eT": eT,
        }
        for i in range(NCORES)
    ]


def run(x, embedding_matrix, W, trace=False, **spmd_kwargs):
    in_maps = _prepare_in_maps(x, embedding_matrix, W)
    nc = _get_nc()
    res = run_bass_kernel_spmd(
        nc, in_maps, core_ids=list(range(NCORES)), trace=trace, **spmd_kwargs
    )
    out = np.concatenate(
        [np.asarray(res.results[i]["out"]) for i in range(NCORES)], axis=0
    )
    return out.astype(np.float32, copy=False), res


def kernel(x, embedding_matrix, W):
    out, _ = run(x, embedding_matrix, W, trace=False)
    return out


# revision 5
# speedup vs baseline: 3.2578x; 1.2942x over previous
"""Trainium2 Bass kernel for out = (x @ W) @ E.T, batch-sharded over 8 NeuronCores.

Shapes (hardcoded, full problem):
  x [4096, 2048] f32, W [2048, 300] f32, E [20000, 300] f32 -> out [4096, 20000] f32

Strategy: data-parallel over batch. Each core gets a 512-row batch shard of x
(pre-transposed on host to xT [2048, 512]), a replicated W [2048, 300], and a
replicated pre-transposed eT [300, 20000]. Host transposes mean every matmul
contracts along the SBUF partition dim with no on-chip transposes.

Per-core device kernel:
  stage 1: xwT[k, b] = sum_i W[i, k] * xT[i, b]     -> [300, 512] kept in SBUF
  stage 2: out[b, c] = sum_k xwT[k, b] * eT[k, c]   -> [512, 20000] streamed out

Inputs are cast to bf16 on the host (halves input DMA, full-rate PE); the
output is written as fp16 (halves output DMA; |out| <= ~200 so fp16 range is
safe and its 11-bit mantissa adds ~2e-4 rel err on top of bf16's ~3e-3).
DMA traffic is split across the three descriptor paths: gpsimd (SWDGE) for
x/W, sync ring for eT streaming, scalar ring for output stores.
"""

import numpy as np

import concourse.bass as bass
import concourse.tile as tile
from concourse import bacc, mybir
from concourse.bass import ts
from concourse.bass_utils import run_bass_kernel_spmd

B, IMG, WORD, NCLS = 4096, 2048, 300, 20000
NCORES = 8
BS = B // NCORES  # 512 batch rows per core

# k (WORD) chunks on the contraction partition dim: 300 = 128 + 128 + 44
KCH = [(0, 128), (128, 128), (256, 44)]
CGRP = 2000  # classes per eT load group
CSUB = 500   # classes per matmul (N <= 512 PSUM-bank limit)

COMPUTE_DT = mybir.dt.bfloat16
OUT_DT = mybir.dt.float16


def build_nc():
    cdt = COMPUTE_DT
    f32 = mybir.dt.float32
    odt = OUT_DT
    nc = bacc.Bacc(
        "TRN2",
        target_bir_lowering=False,
        debug=False,
        num_devices=NCORES,
    )
    xT = nc.declare_dram_parameter("xT", [IMG, BS], cdt, isOutput=False)
    w = nc.declare_dram_parameter("w", [IMG, WORD], cdt, isOutput=False)
    eT = nc.declare_dram_parameter("eT", [WORD, NCLS], cdt, isOutput=False)
    out = nc.declare_dram_parameter("out", [BS, NCLS], odt, isOutput=True)

    NI = IMG // 128  # 16 i-tiles
    S1CH = 4         # stage-1 load chunks (i-tiles per DMA)

    with tile.TileContext(nc) as tc:
        with (
            tc.tile_pool(name="stage1", bufs=1) as s1_pool,
            tc.tile_pool(name="xw", bufs=1) as xw_pool,
            tc.tile_pool(name="et", bufs=3) as et_pool,
            tc.tile_pool(name="outsb", bufs=6) as out_pool,
            tc.tile_pool(name="psum", bufs=8, space="PSUM") as ps_pool,
        ):
            # ---- stage 1: xwT = W^T @ x^T^T  (contract over i=IMG)
            w_sb = s1_pool.tile([128, NI, WORD], cdt)
            x_sb = s1_pool.tile([128, NI, BS], cdt)
            w_r = w.rearrange("(n p) k -> p n k", p=128)
            x_r = xT.rearrange("(n p) b -> p n b", p=128)
            for s in range(0, NI, S1CH):
                nc.gpsimd.dma_start(w_sb[:, s : s + S1CH, :], w_r[:, s : s + S1CH, :])
                nc.gpsimd.dma_start(x_sb[:, s : s + S1CH, :], x_r[:, s : s + S1CH, :])

            xw_sb = []
            for kc, (ko, ks) in enumerate(KCH):
                ps = ps_pool.tile([128, BS], f32, tag="ps", name=f"ps1_{kc}")
                for n in range(NI):
                    nc.tensor.matmul(
                        ps[:ks, :],
                        w_sb[:, n, ko : ko + ks],
                        x_sb[:, n, :],
                        start=(n == 0),
                        stop=(n == NI - 1),
                    )
                t = xw_pool.tile([128, BS], cdt, tag=f"xw{kc}", name=f"xw{kc}")
                nc.vector.tensor_copy(t[:ks, :], ps[:ks, :])
                xw_sb.append(t)

            # ---- stage 2: out = xwT^T @ eT  (contract over k=WORD)
            for g in range(NCLS // CGRP):
                et = et_pool.tile([128, len(KCH), CGRP], cdt, tag="et", name=f"et{g}")
                for kc, (ko, ks) in enumerate(KCH):
                    nc.sync.dma_start(
                        et[:ks, kc, :], eT[ko : ko + ks, g * CGRP : (g + 1) * CGRP]
                    )
                for b in range(BS // 128):
                    ob = out_pool.tile([128, CGRP], odt, tag="ob", name=f"ob_{g}_{b}")
                    pss = [
                        ps_pool.tile([128, CSUB], f32, tag="ps", name=f"ps2_{g}_{b}_{c}")
                        for c in range(CGRP // CSUB)
                    ]
                    # kc outer so the stationary operand is reused across c
                    for kc, (ko, ks) in enumerate(KCH):
                        for c in range(CGRP // CSUB):
                            nc.tensor.matmul(
                                pss[c][:, :],
                                xw_sb[kc][:ks, ts(b, 128)],
                                et[:ks, kc, ts(c, CSUB)],
                                start=(kc == 0),
                                stop=(kc == len(KCH) - 1),
                            )
                    for c in range(CGRP // CSUB):
                        # balance PSUM->SBUF copies: DVE is ~2x ACT, so 3:1
                        if c == 3:
                            nc.scalar.copy(ob[:, ts(c, CSUB)], pss[c][:, :])
                        else:
                            nc.vector.tensor_copy(ob[:, ts(c, CSUB)], pss[c][:, :])
                    nc.scalar.dma_start(
                        out[ts(b, 128), g * CGRP : (g + 1) * CGRP], ob[:]
                    )

    nc.compile()
    return nc


_NC_CACHE = {}


def _get_nc():
    key = (str(COMPUTE_DT), str(OUT_DT))
    if key not in _NC_CACHE:
        _NC_CACHE[key] = build_nc()
    return _NC_CACHE[key]


def _np_dt(dt):
    import ml_dtypes

    if dt == mybir.dt.bfloat16:
        return np.dtype(ml_dtypes.bfloat16)
    if dt == mybir.dt.float16:
        return np.dtype(np.float16)
    return np.dtype(np.float32)


def _prepare_in_maps(x, embedding_matrix, W):
    npdt = _np_dt(COMPUTE_DT)
    x = np.asarray(x, dtype=np.float32)
    E = np.asarray(embedding_matrix, dtype=np.float32)
    Wm = np.asarray(W, dtype=np.float32)
    xT = np.ascontiguousarray(x.T).astype(npdt)  # [IMG, B]
    w = np.ascontiguousarray(Wm).astype(npdt)
    eT = np.ascontiguousarray(E.T).astype(npdt)  # [WORD, NCLS]
    return [
        {
            "xT": np.ascontiguousarray(xT[:, i * BS : (i + 1) * BS]),
            "w": w,
            "eT": eT,
        }
        for i in range(NCORES)
    ]


def run(x, embedding_matrix, W, trace=False, **spmd_kwargs):
    in_maps = _prepare_in_maps(x, embedding_matrix, W)
    nc = _get_nc()
    res = run_bass_kernel_spmd(
        nc, in_maps, core_ids=list(range(NCORES)), trace=trace, **spmd_kwargs
    )
    out = np.concatenate(
        [np.asarray(res.results[i]["out"]) for i in range(NCORES)], axis=0
    )
    return out.astype(np.float32), res


def kernel(x, embedding_matrix, W):
    out, _ = run(x, embedding_matrix, W, trace=False)
    return out


# revision 6
# speedup vs baseline: 3.4573x; 1.0612x over previous
"""Trainium2 Bass kernel for out = (x @ W) @ E.T, batch-sharded over 8 NeuronCores.

Shapes (hardcoded, full problem):
  x [4096, 2048] f32, W [2048, 300] f32, E [20000, 300] f32 -> out [4096, 20000] f32

Strategy: data-parallel over batch. Each core gets a 512-row batch shard of x
(pre-transposed on host to xT [2048, 512]), a replicated W [2048, 300], and a
replicated pre-transposed eT [300, 20000]. Host transposes mean every matmul
contracts along the SBUF partition dim with no on-chip transposes.

Per-core device kernel:
  stage 1: xwT[k, b] = sum_i W[i, k] * xT[i, b]     -> [300, 512] kept in SBUF
  stage 2: out[b, c] = sum_k xwT[k, b] * eT[k, c]   -> [512, 20000] streamed out

Perf notes:
- inputs bf16 (half DMA, full-rate PE), output fp16 (half DMA; |out|<200 and
  fp16's 11-bit mantissa is below bf16 compute noise).
- all input DMAs ride the sync HWDGE ring in FIFO order (x/W chunks first so
  stage 1 starts ~2us in), output stores ride the scalar ring concurrently.
- K=300 tiles as 128+128+44; the two 44-row matmuls of adjacent class chunks
  are packed into disjoint PE row strips (tile_position rows 0-43 / 64-107)
  so they run concurrently -- a K=44 matmul otherwise costs full N cycles.
- PSUM->SBUF copies split 2:2 between DVE and ACT.
"""

import numpy as np

import concourse.bass as bass
import concourse.tile as tile
from concourse import bacc, mybir
from concourse.bass import ts
from concourse.bass_utils import run_bass_kernel_spmd

B, IMG, WORD, NCLS = 4096, 2048, 300, 20000
NCORES = 8
BS = B // NCORES  # 512 batch rows per core

CSUB = 500  # classes per matmul (N <= 512 PSUM-bank limit)
# eT load groups (classes per group): small leading groups so stage 2 can
# start as soon as stage 1 finishes, big groups after.
CGROUPS = [1000, 1000] + [2000] * 9
assert sum(CGROUPS) == NCLS

COMPUTE_DT = mybir.dt.bfloat16
OUT_DT = mybir.dt.float16


def build_nc():
    cdt = COMPUTE_DT
    f32 = mybir.dt.float32
    odt = OUT_DT
    nc = bacc.Bacc(
        "TRN2",
        target_bir_lowering=False,
        debug=False,
        num_devices=NCORES,
    )
    xT = nc.declare_dram_parameter("xT", [IMG, BS], cdt, isOutput=False)
    w = nc.declare_dram_parameter("w", [IMG, WORD], cdt, isOutput=False)
    eT = nc.declare_dram_parameter("eT", [WORD, NCLS], cdt, isOutput=False)
    out = nc.declare_dram_parameter("out", [BS, NCLS], odt, isOutput=True)

    NI = IMG // 128  # 16 i-tiles
    S1CH = 4         # stage-1 load chunks (i-tiles per DMA)
    CGMAX = max(CGROUPS)

    with tile.TileContext(nc) as tc:
        with (
            tc.tile_pool(name="stage1", bufs=1) as s1_pool,
            tc.tile_pool(name="xw", bufs=1) as xw_pool,
            tc.tile_pool(name="et", bufs=3) as et_pool,
            tc.tile_pool(name="outsb", bufs=6) as out_pool,
            tc.tile_pool(name="psum", bufs=8, space="PSUM") as ps_pool,
        ):
            # ---- stage 1 loads: first on the sync FIFO ring, chunked so the
            # PE starts on chunk 0 while later chunks stream.
            w_sb = s1_pool.tile([128, NI, WORD], cdt)
            x_sb = s1_pool.tile([128, NI, BS], cdt)
            w_r = w.rearrange("(n p) k -> p n k", p=128)
            x_r = xT.rearrange("(n p) b -> p n b", p=128)
            for s in range(0, NI, S1CH):
                nc.sync.dma_start(w_sb[:, s : s + S1CH, :], w_r[:, s : s + S1CH, :])
                nc.sync.dma_start(x_sb[:, s : s + S1CH, :], x_r[:, s : s + S1CH, :])

            # ---- stage 1 matmuls: n-outer so each arriving chunk is consumed
            ps1 = [
                ps_pool.tile([128, BS], f32, tag="ps", name=f"ps1_{kc}")
                for kc in range(3)
            ]
            for n in range(NI):
                for kc, (ko, ks) in enumerate([(0, 128), (128, 128), (256, 44)]):
                    nc.tensor.matmul(
                        ps1[kc][:ks, :],
                        w_sb[:, n, ko : ko + ks],
                        x_sb[:, n, :],
                        start=(n == 0),
                        stop=(n == NI - 1),
                    )
            xw_sb = []
            for kc, ks in enumerate([128, 128, 44]):
                t = xw_pool.tile([128, BS], cdt, tag=f"xw{kc}", name=f"xw{kc}")
                nc.vector.tensor_copy(t[:ks, :], ps1[kc][:ks, :])
                xw_sb.append(t)
            # duplicate the K=44 chunk at partitions 64..107 for row-strip
            # packing (SBUF->SBUF DMA shifts partitions; gpsimd ring is idle)
            xw2b = xw_pool.tile([128, BS], cdt, tag="xw2b", name="xw2b")
            nc.gpsimd.dma_start(xw2b[64:108, :], xw_sb[2][:44, :])

            # ---- stage 2: out = xwT^T @ eT  (contract over k=WORD)
            goff = 0
            for g, cg in enumerate(CGROUPS):
                ncs = cg // CSUB  # class sub-chunks in this group
                et = et_pool.tile([128, 4, CGMAX], cdt, tag="et", name=f"et{g}")
                for kc, (ko, ks) in enumerate([(0, 128), (128, 128), (256, 44)]):
                    nc.sync.dma_start(
                        et[:ks, kc, :cg], eT[ko : ko + ks, goff : goff + cg]
                    )
                # K=44 rows again at partitions 64..107 (row-strip B operand)
                nc.sync.dma_start(et[64:108, 3, :cg], eT[256:300, goff : goff + cg])

                for b in range(BS // 128):
                    ob = out_pool.tile([128, CGMAX], odt, tag="ob", name=f"ob_{g}_{b}")
                    pss = [
                        ps_pool.tile([128, CSUB], f32, tag="ps", name=f"ps2_{g}_{b}_{c}")
                        for c in range(ncs)
                    ]
                    for kc in range(2):
                        for c in range(ncs):
                            nc.tensor.matmul(
                                pss[c][:, :],
                                xw_sb[kc][:, ts(b, 128)],
                                et[:, kc, ts(c, CSUB)],
                                start=(kc == 0),
                                stop=False,
                            )
                    # K=44 tail: adjacent class chunks packed into row strips
                    # 0-43 and 64-107 so they execute concurrently.
                    for c in range(ncs):
                        if c % 2 == 0:
                            nc.tensor.matmul(
                                pss[c][:, :],
                                xw_sb[2][:44, ts(b, 128)],
                                et[:44, 2, ts(c, CSUB)],
                                start=False,
                                stop=True,
                                tile_position=(0, 0),
                            )
                        else:
                            nc.tensor.matmul(
                                pss[c][:, :],
                                xw2b[64:108, ts(b, 128)],
                                et[64:108, 3, ts(c, CSUB)],
                                start=False,
                                stop=True,
                                tile_position=(64, 0),
                            )
                    for c in range(ncs):
                        # split PSUM->SBUF copies between DVE and ACT
                        if c % 2 == 0:
                            nc.vector.tensor_copy(ob[:, ts(c, CSUB)], pss[c][:, :])
                        else:
                            nc.scalar.copy(ob[:, ts(c, CSUB)], pss[c][:, :])
                    nc.scalar.dma_start(
                        out[ts(b, 128), goff : goff + cg], ob[:, :cg]
                    )
                goff += cg

    nc.compile()
    return nc


_NC_CACHE = {}


def _get_nc():
    key = (str(COMPUTE_DT), str(OUT_DT))
    if key not in _NC_CACHE:
        _NC_CACHE[key] = build_nc()
    return _NC_CACHE[key]


def _np_dt(dt):
    import ml_dtypes

    if dt == mybir.dt.bfloat16:
        return np.dtype(ml_dtypes.bfloat16)
    if dt == mybir.dt.float16:
        return np.dtype(np.float16)
    return np.dtype(np.float32)


def _prepare_in_maps(x, embedding_matrix, W):
    npdt = _np_dt(COMPUTE_DT)
    x = np.asarray(x, dtype=np.float32)
    E = np.asarray(embedding_matrix, dtype=np.float32)
    Wm = np.asarray(W, dtype=np.float32)
    xT = np.ascontiguousarray(x.T).astype(npdt)  # [IMG, B]
    w = np.ascontiguousarray(Wm).astype(npdt)
    eT = np.ascontiguousarray(E.T).astype(npdt)  # [WORD, NCLS]
    return [
        {
            "xT": np.ascontiguousarray(xT[:, i * BS : (i + 1) * BS]),
            "w": w,
            "eT": eT,
        }
        for i in range(NCORES)
    ]


def run(x, embedding_matrix, W, trace=False, **spmd_kwargs):
    in_maps = _prepare_in_maps(x, embedding_matrix, W)
    nc = _get_nc()
    res = run_bass_kernel_spmd(
        nc, in_maps, core_ids=list(range(NCORES)), trace=trace, **spmd_kwargs
    )
    out = np.concatenate(
        [np.asarray(res.results[i]["out"]) for i in range(NCORES)], axis=0
    )
    return out.astype(np.float32), res


def kernel(x, embedding_matrix, W):
    out, _ = run(x, embedding_matrix, W, trace=False)
    return out


# revision 14
# speedup vs baseline: 3.7056x; 1.0718x over previous
"""Trainium2 Bass kernel for out = (x @ W) @ E.T, batch-sharded over 8 NeuronCores.

Shapes (hardcoded, full problem):
  x [4096, 2048] f32, W [2048, 300] f32, E [20000, 300] f32 -> out [4096, 20000] f32

Strategy: data-parallel over batch. Each core gets a 512-row batch shard of x
(pre-transposed on host to xT [2048, 512]), a replicated W [2048, 300], and a
replicated pre-transposed eT [300, 20000]. Host transposes mean every matmul
contracts along the SBUF partition dim with no on-chip transposes.

Per-core device kernel:
  stage 1: xwT[k, b] = sum_i W[i, k] * xT[i, b]     -> [300, 512] kept in SBUF
  stage 2: out[b, c] = sum_k xwT[k, b] * eT[k, c]   -> [512, 20000] streamed out

Perf notes:
- inputs bf16 (half DMA, full-rate PE), output fp16 (half DMA; |out|<200 and
  fp16's 11-bit mantissa is below bf16 compute noise).
- all input DMAs ride the sync HWDGE ring in FIFO order (x/W chunks first so
  stage 1 starts ~2us in), output stores ride the scalar ring concurrently.
- K=300 tiles as 128+128+44; the two 44-row matmuls of adjacent class chunks
  are packed into disjoint PE row strips (tile_position rows 0-43 / 64-107)
  so they run concurrently -- a K=44 matmul otherwise costs full N cycles.
- PSUM->SBUF copies split 2:2 between DVE and ACT.
"""

import numpy as np

import concourse.bass as bass
import concourse.tile as tile
from concourse import bacc, mybir
from concourse.bass import ts
from concourse.bass_utils import run_bass_kernel_spmd

B, IMG, WORD, NCLS = 4096, 2048, 300, 20000
NCORES = 8
BS = B // NCORES  # 512 batch rows per core

CSUB = 500  # classes per matmul (N <= 512 PSUM-bank limit)
# eT load groups (classes per group): small leading groups so stage 2 can
# start as soon as stage 1 finishes, small trailing group for a short drain.
CGROUPS = [1000, 1000] + [2000] * 8 + [1000, 1000]
assert sum(CGROUPS) == NCLS

COMPUTE_DT = mybir.dt.bfloat16
OUT_DT = mybir.dt.float16


def build_nc():
    cdt = COMPUTE_DT
    f32 = mybir.dt.float32
    odt = OUT_DT
    nc = bacc.Bacc(
        "TRN2",
        target_bir_lowering=False,
        debug=False,
        num_devices=NCORES,
    )
    xT = nc.declare_dram_parameter("xT", [IMG, BS], cdt, isOutput=False)
    w = nc.declare_dram_parameter("w", [IMG, WORD], cdt, isOutput=False)
    eT = nc.declare_dram_parameter("eT", [WORD, NCLS], cdt, isOutput=False)
    out = nc.declare_dram_parameter("out", [BS, NCLS], odt, isOutput=True)

    NI = IMG // 128  # 16 i-tiles
    S1CH = 4         # stage-1 load chunks (i-tiles per DMA)
    CGMAX = max(CGROUPS)

    with tile.TileContext(nc) as tc:
        with (
            tc.tile_pool(name="stage1", bufs=1) as s1_pool,
            tc.tile_pool(name="xw", bufs=1) as xw_pool,
            tc.tile_pool(name="et", bufs=3) as et_pool,
            tc.tile_pool(name="outsb", bufs=6) as out_pool,
            tc.tile_pool(name="psum", bufs=4, space="PSUM") as ps_pool,
        ):
            # ---- PE warmup: ~4us of tiny matmuls on junk data so the HAM
            # clock gate reaches 8/8 before the real matmuls begin.
            wu = s1_pool.tile([128, 64], cdt, name="warmup")
            nc.gpsimd.memset(wu[:], 0.0)
            wups = ps_pool.tile([128, 64], f32, tag="ps", name="wups")
            NWU = 56
            for i in range(NWU):
                nc.tensor.matmul(
                    wups[:64, :], wu[:, :64], wu[:, :],
                    start=(i == 0), stop=(i == NWU - 1),
                )

            # ---- stage 1 loads: first on the sync FIFO ring, chunked so the
            # PE starts on chunk 0 while later chunks stream.
            w_sb = s1_pool.tile([128, NI, WORD], cdt)
            x_sb = s1_pool.tile([128, NI, BS], cdt)
            w_r = w.rearrange("(n p) k -> p n k", p=128)
            x_r = xT.rearrange("(n p) b -> p n b", p=128)
            for s in range(0, NI, S1CH):
                nc.sync.dma_start(w_sb[:, s : s + S1CH, :], w_r[:, s : s + S1CH, :])
                nc.sync.dma_start(x_sb[:, s : s + S1CH, :], x_r[:, s : s + S1CH, :])

            # ---- stage 1 matmuls: n-outer so each arriving chunk is consumed
            ps1 = [
                ps_pool.tile([128, BS], f32, tag="ps", name=f"ps1_{kc}")
                for kc in range(3)
            ]
            for n in range(NI):
                for kc, (ko, ks) in enumerate([(0, 128), (128, 128), (256, 44)]):
                    nc.tensor.matmul(
                        ps1[kc][:ks, :],
                        w_sb[:, n, ko : ko + ks],
                        x_sb[:, n, :],
                        start=(n == 0),
                        stop=(n == NI - 1),
                    )
            xw_sb = []
            for kc, ks in enumerate([128, 128, 44]):
                t = xw_pool.tile([128, BS], cdt, tag=f"xw{kc}", name=f"xw{kc}")
                nc.vector.tensor_copy(t[:ks, :], ps1[kc][:ks, :])
                xw_sb.append(t)
            # duplicate the K=44 chunk at partitions 64..107 for row-strip
            # packing (SBUF->SBUF DMA shifts partitions; gpsimd ring is idle)
            xw2b = xw_pool.tile([128, BS], cdt, tag="xw2b", name="xw2b")
            nc.gpsimd.dma_start(xw2b[64:108, :], xw_sb[2][:44, :])

            # ---- stage 2: out = xwT^T @ eT  (contract over k=WORD)
            goff = 0
            for g, cg in enumerate(CGROUPS):
                ncs = cg // CSUB  # class sub-chunks in this group
                et = et_pool.tile([128, 4, CGMAX], cdt, tag="et", name=f"et{g}")
                for kc, (ko, ks) in enumerate([(0, 128), (128, 128), (256, 44)]):
                    nc.sync.dma_start(
                        et[:ks, kc, :cg], eT[ko : ko + ks, goff : goff + cg]
                    )
                # K=44 rows again at partitions 64..107 (row-strip B operand)
                nc.sync.dma_start(et[64:108, 3, :cg], eT[256:300, goff : goff + cg])

                for b in range(BS // 128):
                    ob = out_pool.tile([128, CGMAX], odt, tag="ob", name=f"ob_{g}_{b}")
                    npair = ncs // 2
                    pss = [
                        ps_pool.tile(
                            [128, 2, 512], f32, tag="ps", name=f"ps2_{g}_{b}_{p}"
                        )
                        for p in range(npair)
                    ]
                    for kc in range(2):
                        for c in range(ncs):
                            nc.tensor.matmul(
                                pss[c // 2][:, c % 2, :CSUB],
                                xw_sb[kc][:, ts(b, 128)],
                                et[:, kc, ts(c, CSUB)],
                                start=(kc == 0),
                                stop=False,
                            )
                    # K=44 tail: adjacent class chunks packed into row strips
                    # 0-43 and 64-107 so they execute concurrently.
                    for c in range(ncs):
                        if c % 2 == 0:
                            nc.tensor.matmul(
                                pss[c // 2][:, c % 2, :CSUB],
                                xw_sb[2][:44, ts(b, 128)],
                                et[:44, 2, ts(c, CSUB)],
                                start=False,
                                stop=True,
                                tile_position=(0, 0),
                            )
                        else:
                            nc.tensor.matmul(
                                pss[c // 2][:, c % 2, :CSUB],
                                xw2b[64:108, ts(b, 128)],
                                et[64:108, 3, ts(c, CSUB)],
                                start=False,
                                stop=True,
                                tile_position=(64, 0),
                            )
                    # one PSUM->SBUF copy per c-pair, split DVE/ACT
                    for p in range(npair):
                        src_ap = pss[p][:, :, :CSUB]
                        dst_ap = ob[:, 2 * p * CSUB : 2 * (p + 1) * CSUB]
                        if (b + p) % 2 == 0:
                            nc.vector.tensor_copy(dst_ap, src_ap)
                        else:
                            nc.scalar.copy(dst_ap, src_ap)
                    nc.scalar.dma_start(
                        out[ts(b, 128), goff : goff + cg], ob[:, :cg]
                    )
                goff += cg

    nc.compile()
    return nc


_NC_CACHE = {}


def _get_nc():
    key = (str(COMPUTE_DT), str(OUT_DT))
    if key not in _NC_CACHE:
        _NC_CACHE[key] = build_nc()
    return _NC_CACHE[key]


def _np_dt(dt):
    import ml_dtypes

    if dt == mybir.dt.bfloat16:
        return np.dtype(ml_dtypes.bfloat16)
    if dt == mybir.dt.float16:
        return np.dtype(np.float16)
    return np.dtype(np.float32)


def _prepare_in_maps(x, embedding_matrix, W):
    npdt = _np_dt(COMPUTE_DT)
    x = np.asarray(x, dtype=np.float32)
    E = np.asarray(embedding_matrix, dtype=np.float32)
    Wm = np.asarray(W, dtype=np.float32)
    xT = np.ascontiguousarray(x.T).astype(npdt)  # [IMG, B]
    w = np.ascontiguousarray(Wm).astype(npdt)
    eT = np.ascontiguousarray(E.T).astype(npdt)  # [WORD, NCLS]
    return [
        {
            "xT": np.ascontiguousarray(xT[:, i * BS : (i + 1) * BS]),
            "w": w,
            "eT": eT,
        }
        for i in range(NCORES)
    ]


def run(x, embedding_matrix, W, trace=False, **spmd_kwargs):
    in_maps = _prepare_in_maps(x, embedding_matrix, W)
    nc = _get_nc()
    res = run_bass_kernel_spmd(
        nc, in_maps, core_ids=list(range(NCORES)), trace=trace, **spmd_kwargs
    )
    out = np.concatenate(
        [np.asarray(res.results[i]["out"]) for i in range(NCORES)], axis=0
    )
    return out.astype(np.float32), res


def kernel(x, embedding_matrix, W):
    out, _ = run(x, embedding_matrix, W, trace=False)
    return out
